# revision 6
# baseline (speedup 1.0000x reference)
"""Trainium2 Bass kernel for coverage-attention (pointer-generator style).

Sharding: data-parallel over batch B=8 across 8 NeuronCores (1 element
per core, zero collectives).

Device math per core (batch element b), H=768 on partitions (6 tiles),
S=256 on the free axis:
  EFT[h,s]  = (W_h @ enc^T)            setup, bf16 matmuls, f32 PSUM
  dfT[h,t]  = (W_dec @ dec^T + b_dec)  setup
  scan over t (recurrence on cov only):
    cb   = ones x cov + r_prev x exp_prev   (TensorE outer products, PSUM)
    x    = cb*w_c[h] + EFD[h,s]             (VectorE stt, bf16; EFD=EFT+dec_t
                                             precomputed on GPSIMD off-chain)
    th   = tanh(x)                          (ScalarE, pair-merged FD=512)
    sc   = sum_h v[h]*th[h,s]               (TensorE matvec, f32 PSUM)
    exp  = Exp(sc) -> bf16, denom via accum (ScalarE)
    r    = 1/denom; r_row = ones*r          (VectorE)
    cov  = exp*r + cov                      (VectorE, bf16 state, off-chain)
    DMA exp row (bf16, unnormalized) to DRAM
  end: read back exp matrix, transpose via TensorE, ht_unnorm = exp^T @ enc.

Host: normalizes exp rows (float64) -> attn_dist, scales ht_unnorm rows,
derives coverage_final and the coverage loss from attn_dist.
"""

import functools
import sys

import numpy as np

sys.path.insert(0, "/opt/trn_rl_repo")

from concourse import bacc, bass, mybir, tile  # noqa: E402
from concourse.bass_utils import run_bass_kernel_spmd  # noqa: E402

B, T, S, H = 8, 64, 256, 768
HT = H // 128  # 6 h-tiles
ST = S // 128  # 2 s-tiles
F32 = mybir.dt.float32
BF16 = mybir.dt.bfloat16
AF = mybir.ActivationFunctionType
ALU = mybir.AluOpType


def build_graph():
    nc = bacc.Bacc(None, target_bir_lowering=False, debug=False)

    enc_d = nc.dram_tensor("enc", [S, H], F32, kind="ExternalInput")
    encT_d = nc.dram_tensor("encT", [H, S], F32, kind="ExternalInput")
    WhT_d = nc.dram_tensor("WhT", [H, H], F32, kind="ExternalInput")
    WdT_d = nc.dram_tensor("WdT", [H, H], F32, kind="ExternalInput")
    decT_d = nc.dram_tensor("decT", [H, T], F32, kind="ExternalInput")
    bdec_d = nc.dram_tensor("bdec", [128, HT], F32, kind="ExternalInput")
    vcol_d = nc.dram_tensor("vcol", [128, HT], F32, kind="ExternalInput")
    wc_d = nc.dram_tensor("wc", [128, HT], F32, kind="ExternalInput")
    cov_d = nc.dram_tensor("cov0", [1, S], F32, kind="ExternalInput")
    eye_d = nc.dram_tensor("eye64", [64, 64], F32, kind="ExternalInput")

    ht_d = nc.dram_tensor("ht", [T, H], F32, kind="ExternalOutput")
    expd_d = nc.dram_tensor("expd", [T, S], BF16, kind="ExternalOutput")

    with tile.TileContext(nc) as tc:
        with (
            tc.tile_pool(name="const", bufs=1) as cp,
            tc.tile_pool(name="xw", bufs=4) as xp,
            tc.tile_pool(name="tw", bufs=4) as tp,
            tc.tile_pool(name="rows", bufs=4) as rp,
            tc.tile_pool(name="ps_setup", bufs=2, space="PSUM") as pset,
            tc.tile_pool(name="ps_cb", bufs=2, space="PSUM") as pcb,
            tc.tile_pool(name="ps_sc", bufs=2, space="PSUM") as psc,
        ):
            # ---- constant loads + bf16 casts ------------------------------
            WhT = cp.tile([128, HT, H], F32)
            nc.sync.dma_start(WhT[:], WhT_d.rearrange("(a p) h -> p a h", p=128))
            WdT = cp.tile([128, HT, H], F32)
            nc.sync.dma_start(WdT[:], WdT_d.rearrange("(a p) h -> p a h", p=128))
            encT = cp.tile([128, HT, S], F32)
            nc.sync.dma_start(encT[:], encT_d.rearrange("(a p) s -> p a s", p=128))
            enc = cp.tile([128, ST, H], F32)
            nc.sync.dma_start(enc[:], enc_d.rearrange("(a p) h -> p a h", p=128))
            decT = cp.tile([128, HT, T], F32)
            nc.sync.dma_start(decT[:], decT_d.rearrange("(a p) t -> p a t", p=128))
            bdec = cp.tile([128, HT], F32)
            nc.sync.dma_start(bdec[:], bdec_d[:])
            vcol_f = cp.tile([128, HT], F32)
            nc.sync.dma_start(vcol_f[:], vcol_d[:])
            wc = cp.tile([128, HT], F32)
            nc.sync.dma_start(wc[:], wc_d[:])
            eye_f = cp.tile([64, 64], F32)
            nc.sync.dma_start(eye_f[:], eye_d[:])
            cov_f = cp.tile([1, S], F32)
            nc.sync.dma_start(cov_f[:], cov_d[:])

            WhT_b = cp.tile([128, HT, H], BF16)
            WdT_b = cp.tile([128, HT, H], BF16)
            encT_b = cp.tile([128, HT, S], BF16)
            enc_b = cp.tile([128, ST, H], BF16)
            decT_b = cp.tile([128, HT, T], BF16)
            vcol = cp.tile([128, HT], BF16)
            for i in range(HT):
                nc.vector.tensor_copy(WhT_b[:, i, :], WhT[:, i, :])
                nc.vector.tensor_copy(WdT_b[:, i, :], WdT[:, i, :])
                nc.vector.tensor_copy(encT_b[:, i, :], encT[:, i, :])
                nc.vector.tensor_copy(decT_b[:, i, :], decT[:, i, :])
            for i in range(ST):
                nc.vector.tensor_copy(enc_b[:, i, :], enc[:, i, :])
            nc.vector.tensor_copy(vcol[:], vcol_f[:])

            ones_b = cp.tile([1, 128], BF16)
            nc.vector.memset(ones_b[:], 1.0)

            # coverage state (bf16 row), double-buffered across steps
            cov0 = cp.tile([1, S], BF16)
            cov1 = cp.tile([1, S], BF16)
            nc.vector.tensor_copy(cov0[:], cov_f[:])
            cov_tiles = [cov0, cov1]

            # ---- enc_feature^T = W_h @ enc^T  [h,s] (f32 result) ----------
            EFT = cp.tile([128, HT, S], F32)
            for hm in range(HT):
                ps = pset.tile([128, S], F32, tag="pset")
                for kt in range(HT):
                    nc.tensor.matmul(
                        ps[:],
                        WhT_b[:, kt, hm * 128 : (hm + 1) * 128],
                        encT_b[:, kt, :],
                        start=(kt == 0),
                        stop=(kt == HT - 1),
                    )
                nc.scalar.copy(EFT[:, hm, :], ps[:])

            # ---- dec_fea^T = W_dec @ dec^T + b_dec  [h,t] (f32) -----------
            dfT = cp.tile([128, HT, T], F32)
            for hm in range(HT):
                ps = pset.tile([128, T], F32, tag="pset")
                for kt in range(HT):
                    nc.tensor.matmul(
                        ps[:],
                        WdT_b[:, kt, hm * 128 : (hm + 1) * 128],
                        decT_b[:, kt, :],
                        start=(kt == 0),
                        stop=(kt == HT - 1),
                    )
                nc.scalar.activation(
                    dfT[:, hm, :], ps[:], AF.Identity, bias=bdec[:, hm : hm + 1]
                )

            # EFD(t) = EFT + dec_fea[:,t], bf16, double-buffered (GPSIMD)
            efd0 = cp.tile([128, HT, S], BF16)
            efd1 = cp.tile([128, HT, S], BF16)
            efd_tiles = [efd0, efd1]
            for hm in range(HT):
                nc.gpsimd.tensor_scalar(
                    out=efd0[:, hm, :],
                    in0=EFT[:, hm, :],
                    scalar1=dfT[:, hm, 0:1],
                    scalar2=None,
                    op0=ALU.add,
                )

            # ---- the sequential coverage scan -----------------------------
            r_row_prev = None
            exp_prev = None
            for t in range(T):
                cov_in = cov_tiles[t % 2]
                cov_out = cov_tiles[(t + 1) % 2]
                efd = efd_tiles[t % 2]
                efd_next = efd_tiles[(t + 1) % 2]

                # cb = ones x cov (+ r_prev x exp_prev), f32 PSUM
                cb = pcb.tile([128, S], F32, tag="cb")
                if t == 0:
                    nc.tensor.matmul(
                        cb[:], ones_b[0:1, :], cov_in[0:1, :], start=True, stop=True
                    )
                else:
                    nc.tensor.matmul(
                        cb[:], ones_b[0:1, :], cov_in[0:1, :], start=True, stop=False
                    )
                    nc.tensor.matmul(
                        cb[:],
                        r_row_prev[0:1, :],
                        exp_prev[0:1, :],
                        start=False,
                        stop=True,
                    )
                cbs = xp.tile([128, S], BF16, tag="cbs")
                nc.vector.tensor_copy(cbs[:], cb[:])

                xball = xp.tile([128, HT, S], BF16, tag="xb")
                for hm in range(HT):
                    nc.vector.scalar_tensor_tensor(
                        out=xball[:, hm, :],
                        in0=cbs[:],
                        scalar=wc[:, hm : hm + 1],
                        in1=efd[:, hm, :],
                        op0=ALU.mult,
                        op1=ALU.add,
                    )
                tball = tp.tile([128, HT, S], BF16, tag="tb")
                sc = psc.tile([1, S], F32)
                for g in range(HT // 2):
                    nc.scalar.activation(
                        tball[:, 2 * g : 2 * g + 2, :],
                        xball[:, 2 * g : 2 * g + 2, :],
                        AF.Tanh,
                    )
                    for hm in (2 * g, 2 * g + 1):
                        nc.tensor.matmul(
                            sc[:],
                            vcol[:, hm : hm + 1],
                            tball[:, hm, :],
                            start=(hm == 0),
                            stop=(hm == HT - 1),
                        )

                # EFD for next step, off-chain on GPSIMD
                if t + 1 < T:
                    for hm in range(HT):
                        nc.gpsimd.tensor_scalar(
                            out=efd_next[:, hm, :],
                            in0=EFT[:, hm, :],
                            scalar1=dfT[:, hm, t + 1 : t + 2],
                            scalar2=None,
                            op0=ALU.add,
                        )

                exp_row = rp.tile([1, S], BF16, tag="exp")
                denom = rp.tile([1, 1], F32, tag="den")
                recip = rp.tile([1, 1], F32, tag="rec")
                r_row = rp.tile([1, 128], BF16, tag="rr")
                nc.scalar.activation(exp_row[:], sc[:], AF.Exp, accum_out=denom[:])
                nc.vector.reciprocal(recip[:], denom[:])
                # r_row = ones * r (bf16 row for next step's outer product)
                nc.vector.tensor_scalar(
                    out=r_row[:],
                    in0=ones_b[:],
                    scalar1=recip[0:1, 0:1],
                    scalar2=None,
                    op0=ALU.mult,
                )
                # cov state update (off-chain): cov += exp * r
                nc.vector.scalar_tensor_tensor(
                    out=cov_out[:],
                    in0=exp_row[:],
                    scalar=recip[0:1, 0:1],
                    in1=cov_in[:],
                    op0=ALU.mult,
                    op1=ALU.add,
                )
                nc.sync.dma_start(expd_d[t : t + 1, :], exp_row[:])
                r_row_prev = r_row
                exp_prev = exp_row

            # ---- ht_unnorm = exp^T @ enc for all steps --------------------
            exp_back = cp.tile([64, S], BF16)
            nc.sync.dma_start(exp_back[:], expd_d[:])
            exp_back_f = cp.tile([64, S], F32)
            nc.vector.tensor_copy(exp_back_f[:], exp_back[:])
            exp_cols = cp.tile([128, ST, 64], BF16)
            for si in range(ST):
                pt = pset.tile([128, 64], F32, tag="pset")
                nc.tensor.transpose(
                    pt[:], exp_back_f[:, si * 128 : (si + 1) * 128], eye_f[:]
                )
                nc.vector.tensor_copy(exp_cols[:, si, :], pt[:])

            hts = cp.tile([64, H], F32)
            for n0, n1 in ((0, 512), (512, 768)):
                ph = pset.tile([64, n1 - n0], F32, tag="pset")
                for si in range(ST):
                    nc.tensor.matmul(
                        ph[:],
                        exp_cols[:, si, :],
                        enc_b[:, si, n0:n1],
                        start=(si == 0),
                        stop=(si == ST - 1),
                    )
                nc.scalar.copy(hts[:, n0:n1], ph[:])
            nc.sync.dma_start(ht_d[:], hts[:])

    nc.compile()
    return nc


@functools.lru_cache(maxsize=1)
def _graph():
    return build_graph()


def make_in_maps(
    decoder_outputs, encoder_outputs, coverage, W_h, W_dec, b_dec, w_c, v
):
    f = np.float32
    WhT = np.ascontiguousarray(W_h.T)
    WdT = np.ascontiguousarray(W_dec.T)
    bdec_c = np.ascontiguousarray(b_dec.reshape(HT, 128).T)
    vcol_c = np.ascontiguousarray(v.reshape(HT, 128).T)
    wc_c = np.ascontiguousarray(w_c.reshape(HT, 128).T)
    eye = np.eye(64, dtype=f)
    in_maps = []
    for b in range(B):
        in_maps.append(
            {
                "enc": np.ascontiguousarray(encoder_outputs[b]),
                "encT": np.ascontiguousarray(encoder_outputs[b].T),
                "WhT": WhT,
                "WdT": WdT,
                "decT": np.ascontiguousarray(decoder_outputs[b].T),
                "bdec": bdec_c,
                "vcol": vcol_c,
                "wc": wc_c,
                "cov0": np.ascontiguousarray(coverage[b].reshape(1, S)),
                "eye64": eye,
            }
        )
    return in_maps


def kernel(
    decoder_outputs,
    decoder_input_mask,
    encoder_outputs,
    enc_padding_mask,
    coverage,
    W_h,
    W_dec,
    b_dec,
    w_c,
    v,
):
    f = np.float32
    decoder_outputs = np.asarray(decoder_outputs, f)
    decoder_input_mask = np.asarray(decoder_input_mask, f)
    encoder_outputs = np.asarray(encoder_outputs, f)
    coverage = np.asarray(coverage, f)
    W_h = np.asarray(W_h, f)
    W_dec = np.asarray(W_dec, f)
    b_dec = np.asarray(b_dec, f)
    w_c = np.asarray(w_c, f)
    v = np.asarray(v, f)

    in_maps = make_in_maps(
        decoder_outputs, encoder_outputs, coverage, W_h, W_dec, b_dec, w_c, v
    )
    nc = _graph()
    res = run_bass_kernel_spmd(nc, in_maps, core_ids=list(range(B)))
    results = res.results

    # host-side normalization (float64): attn = exp / sum(exp)
    exp_all = np.stack(
        [results[b]["expd"] for b in range(B)]
    ).astype(np.float64)  # [B,T,S]
    denom = exp_all.sum(-1, keepdims=True)
    attn64 = exp_all / denom
    attn_dist = attn64.astype(f)
    ht_un = np.stack([results[b]["ht"] for b in range(B)]).astype(np.float64)
    ht_hat = (ht_un / denom).astype(f)

    cov0_64 = coverage.astype(np.float64)
    csum = np.cumsum(attn64, axis=1)
    cov_before = cov0_64[:, None, :] + csum - attn64
    coverage_final = (cov0_64 + csum[:, -1, :]).astype(f)
    step_losses = np.minimum(attn64, cov_before).sum(-1)  # [B,T]
    mask64 = decoder_input_mask.astype(np.float64)
    converge_loss = np.float32((step_losses * mask64).sum() / mask64.sum())

    return ht_hat, attn_dist, converge_loss, coverage_final


# revision 7
# speedup vs baseline: 4.6593x; 4.6593x over previous
"""Trainium2 Bass kernel for coverage-attention (pointer-generator style).

Sharding: data-parallel over batch B=8 across 8 NeuronCores (1 element
per core, zero collectives).

Device math per core (batch element b), H=768 on partitions (6 tiles),
S=256 on the free axis:
  EFT[h,s]  = (W_h @ enc^T)            setup, bf16 matmuls, f32 PSUM
  dfT[h,t]  = (W_dec @ dec^T + b_dec)  setup
  scan over t (recurrence on cov only):
    cb   = ones x cov + r_prev x exp_prev   (TensorE outer products, PSUM)
    x    = cb*w_c[h] + EFD[h,s]             (VectorE stt, bf16; EFD=EFT+dec_t
                                             precomputed on GPSIMD off-chain)
    th   = tanh(x)                          (ScalarE, pair-merged FD=512)
    sc   = sum_h v[h]*th[h,s]               (TensorE matvec, f32 PSUM)
    exp  = Exp(sc) -> bf16, denom via accum (ScalarE)
    r    = 1/denom; r_row = ones*r          (VectorE)
    cov  = exp*r + cov                      (VectorE, bf16 state, off-chain)
    DMA exp row (bf16, unnormalized) to DRAM
  end: read back exp matrix, transpose via TensorE, ht_unnorm = exp^T @ enc.

Host: normalizes exp rows (float64) -> attn_dist, scales ht_unnorm rows,
derives coverage_final and the coverage loss from attn_dist.
"""

import functools
import sys

import numpy as np

sys.path.insert(0, "/opt/trn_rl_repo")

from concourse import bacc, bass, mybir, tile  # noqa: E402
from concourse.bass_utils import run_bass_kernel_spmd  # noqa: E402

B, T, S, H = 8, 64, 256, 768
HT = H // 128  # 6 h-tiles
ST = S // 128  # 2 s-tiles
F32 = mybir.dt.float32
BF16 = mybir.dt.bfloat16
AF = mybir.ActivationFunctionType
ALU = mybir.AluOpType


def build_graph():
    nc = bacc.Bacc(None, target_bir_lowering=False, debug=False)

    enc_d = nc.dram_tensor("enc", [S, H], F32, kind="ExternalInput")
    encT_d = nc.dram_tensor("encT", [H, S], F32, kind="ExternalInput")
    WhT_d = nc.dram_tensor("WhT", [H, H], F32, kind="ExternalInput")
    WdT_d = nc.dram_tensor("WdT", [H, H], F32, kind="ExternalInput")
    decT_d = nc.dram_tensor("decT", [H, T], F32, kind="ExternalInput")
    bdec_d = nc.dram_tensor("bdec", [128, HT], F32, kind="ExternalInput")
    vcol_d = nc.dram_tensor("vcol", [128, HT], F32, kind="ExternalInput")
    wc_d = nc.dram_tensor("wc", [128, HT], F32, kind="ExternalInput")
    cov_d = nc.dram_tensor("cov0", [1, S], F32, kind="ExternalInput")
    eye_d = nc.dram_tensor("eye64", [64, 64], F32, kind="ExternalInput")

    ht_d = nc.dram_tensor("ht", [T, H], F32, kind="ExternalOutput")
    expd_d = nc.dram_tensor("expd", [T, S], BF16, kind="ExternalOutput")

    with tile.TileContext(nc) as tc:
        with (
            tc.tile_pool(name="const", bufs=1) as cp,
            tc.tile_pool(name="xw", bufs=4) as xp,
            tc.tile_pool(name="tw", bufs=4) as tp,
            tc.tile_pool(name="rows", bufs=4) as rp,
            tc.tile_pool(name="ps_setup", bufs=2, space="PSUM") as pset,
            tc.tile_pool(name="ps_cb", bufs=2, space="PSUM") as pcb,
            tc.tile_pool(name="ps_sc", bufs=2, space="PSUM") as psc,
        ):
            # ---- constant loads + bf16 casts ------------------------------
            WhT = cp.tile([128, HT, H], F32)
            nc.sync.dma_start(WhT[:], WhT_d.rearrange("(a p) h -> p a h", p=128))
            WdT = cp.tile([128, HT, H], F32)
            nc.sync.dma_start(WdT[:], WdT_d.rearrange("(a p) h -> p a h", p=128))
            encT = cp.tile([128, HT, S], F32)
            nc.sync.dma_start(encT[:], encT_d.rearrange("(a p) s -> p a s", p=128))
            enc = cp.tile([128, ST, H], F32)
            nc.sync.dma_start(enc[:], enc_d.rearrange("(a p) h -> p a h", p=128))
            decT = cp.tile([128, HT, T], F32)
            nc.sync.dma_start(decT[:], decT_d.rearrange("(a p) t -> p a t", p=128))
            bdec = cp.tile([128, HT], F32)
            nc.sync.dma_start(bdec[:], bdec_d[:])
            vcol_f = cp.tile([128, HT], F32)
            nc.sync.dma_start(vcol_f[:], vcol_d[:])
            wc = cp.tile([128, HT], F32)
            nc.sync.dma_start(wc[:], wc_d[:])
            eye_f = cp.tile([64, 64], F32)
            nc.sync.dma_start(eye_f[:], eye_d[:])
            cov_f = cp.tile([1, S], F32)
            nc.sync.dma_start(cov_f[:], cov_d[:])

            WhT_b = cp.tile([128, HT, H], BF16)
            WdT_b = cp.tile([128, HT, H], BF16)
            encT_b = cp.tile([128, HT, S], BF16)
            enc_b = cp.tile([128, ST, H], BF16)
            decT_b = cp.tile([128, HT, T], BF16)
            vcol = cp.tile([128, HT], BF16)
            for i in range(HT):
                nc.vector.tensor_copy(WhT_b[:, i, :], WhT[:, i, :])
                nc.vector.tensor_copy(WdT_b[:, i, :], WdT[:, i, :])
                nc.vector.tensor_copy(encT_b[:, i, :], encT[:, i, :])
                nc.vector.tensor_copy(decT_b[:, i, :], decT[:, i, :])
            for i in range(ST):
                nc.vector.tensor_copy(enc_b[:, i, :], enc[:, i, :])
            nc.vector.tensor_copy(vcol[:], vcol_f[:])

            ones_b = cp.tile([1, 128], BF16)
            nc.vector.memset(ones_b[:], 1.0)

            # coverage state (bf16 row), double-buffered across steps
            cov0 = cp.tile([1, S], BF16)
            cov1 = cp.tile([1, S], BF16)
            nc.vector.tensor_copy(cov0[:], cov_f[:])
            cov_tiles = [cov0, cov1]

            # ---- enc_feature^T = W_h @ enc^T  [h,s] (f32 result) ----------
            EFT = cp.tile([128, HT, S], F32)
            for hm in range(HT):
                ps = pset.tile([128, S], F32, tag="pset")
                for kt in range(HT):
                    nc.tensor.matmul(
                        ps[:],
                        WhT_b[:, kt, hm * 128 : (hm + 1) * 128],
                        encT_b[:, kt, :],
                        start=(kt == 0),
                        stop=(kt == HT - 1),
                    )
                nc.scalar.copy(EFT[:, hm, :], ps[:])

            # ---- dec_fea^T = W_dec @ dec^T + b_dec  [h,t] (f32) -----------
            dfT = cp.tile([128, HT, T], F32)
            for hm in range(HT):
                ps = pset.tile([128, T], F32, tag="pset")
                for kt in range(HT):
                    nc.tensor.matmul(
                        ps[:],
                        WdT_b[:, kt, hm * 128 : (hm + 1) * 128],
                        decT_b[:, kt, :],
                        start=(kt == 0),
                        stop=(kt == HT - 1),
                    )
                nc.scalar.activation(
                    dfT[:, hm, :], ps[:], AF.Identity, bias=bdec[:, hm : hm + 1]
                )

            # EFT in bf16 for the per-step adds
            EFT_b = cp.tile([128, HT, S], BF16)
            for hm in range(HT):
                nc.vector.tensor_copy(EFT_b[:, hm, :], EFT[:, hm, :])

            # ---- the sequential coverage scan -----------------------------
            # cov state lives as row 0 of the previous step's broadcast
            # (cbs row p == cov for every p), so no separate cov tile.
            r_row_prev = None
            exp_prev = None
            cbs_prev = None
            for t in range(T):
                # cb = ones x cov (+ r_prev x exp_prev), f32 PSUM
                cb = pcb.tile([128, S], F32, tag="cb")
                cov_src = cov_tiles[0] if t == 0 else cbs_prev
                if t == 0:
                    nc.tensor.matmul(
                        cb[:], ones_b[0:1, :], cov_src[0:1, :], start=True, stop=True
                    )
                else:
                    nc.tensor.matmul(
                        cb[:], ones_b[0:1, :], cov_src[0:1, :], start=True, stop=False
                    )
                    nc.tensor.matmul(
                        cb[:],
                        r_row_prev[0:1, :],
                        exp_prev[0:1, :],
                        start=False,
                        stop=True,
                    )
                cbs = xp.tile([128, S], BF16, tag="cbs")
                nc.vector.tensor_copy(cbs[:], cb[:])

                xball = xp.tile([128, HT, S], BF16, tag="xb")
                for hm in range(HT):
                    nc.vector.scalar_tensor_tensor(
                        out=xball[:, hm, :],
                        in0=cbs[:],
                        scalar=wc[:, hm : hm + 1],
                        in1=EFT_b[:, hm, :],
                        op0=ALU.mult,
                        op1=ALU.add,
                    )
                tball = tp.tile([128, HT, S], BF16, tag="tb")
                sc = psc.tile([1, S], F32)
                for hm in range(HT):
                    nc.scalar.activation(
                        tball[:, hm, :],
                        xball[:, hm, :],
                        AF.Tanh,
                        bias=dfT[:, hm, t : t + 1],
                    )
                    nc.tensor.matmul(
                        sc[:],
                        vcol[:, hm : hm + 1],
                        tball[:, hm, :],
                        start=(hm == 0),
                        stop=(hm == HT - 1),
                    )

                exp_row = rp.tile([1, S], BF16, tag="exp")
                denom = rp.tile([1, 1], F32, tag="den")
                recip = rp.tile([1, 1], F32, tag="rec")
                r_row = rp.tile([1, 128], BF16, tag="rr")
                nc.scalar.activation(exp_row[:], sc[:], AF.Exp, accum_out=denom[:])
                nc.vector.reciprocal(recip[:], denom[:])
                # r_row = ones * r (bf16 row for next step's outer product)
                nc.vector.tensor_scalar(
                    out=r_row[:],
                    in0=ones_b[:],
                    scalar1=recip[0:1, 0:1],
                    scalar2=None,
                    op0=ALU.mult,
                )
                nc.sync.dma_start(expd_d[t : t + 1, :], exp_row[:])
                r_row_prev = r_row
                exp_prev = exp_row
                cbs_prev = cbs

            # ---- ht_unnorm = exp^T @ enc for all steps --------------------
            exp_back = cp.tile([64, S], BF16)
            nc.sync.dma_start(exp_back[:], expd_d[:])
            exp_back_f = cp.tile([64, S], F32)
            nc.vector.tensor_copy(exp_back_f[:], exp_back[:])
            exp_cols = cp.tile([128, ST, 64], BF16)
            for si in range(ST):
                pt = pset.tile([128, 64], F32, tag="pset")
                nc.tensor.transpose(
                    pt[:], exp_back_f[:, si * 128 : (si + 1) * 128], eye_f[:]
                )
                nc.vector.tensor_copy(exp_cols[:, si, :], pt[:])

            hts = cp.tile([64, H], F32)
            for n0, n1 in ((0, 512), (512, 768)):
                ph = pset.tile([64, n1 - n0], F32, tag="pset")
                for si in range(ST):
                    nc.tensor.matmul(
                        ph[:],
                        exp_cols[:, si, :],
                        enc_b[:, si, n0:n1],
                        start=(si == 0),
                        stop=(si == ST - 1),
                    )
                nc.scalar.copy(hts[:, n0:n1], ph[:])
            nc.sync.dma_start(ht_d[:], hts[:])

    nc.compile()
    return nc


@functools.lru_cache(maxsize=1)
def _graph():
    return build_graph()


def make_in_maps(
    decoder_outputs, encoder_outputs, coverage, W_h, W_dec, b_dec, w_c, v
):
    f = np.float32
    WhT = np.ascontiguousarray(W_h.T)
    WdT = np.ascontiguousarray(W_dec.T)
    bdec_c = np.ascontiguousarray(b_dec.reshape(HT, 128).T)
    vcol_c = np.ascontiguousarray(v.reshape(HT, 128).T)
    wc_c = np.ascontiguousarray(w_c.reshape(HT, 128).T)
    eye = np.eye(64, dtype=f)
    in_maps = []
    for b in range(B):
        in_maps.append(
            {
                "enc": np.ascontiguousarray(encoder_outputs[b]),
                "encT": np.ascontiguousarray(encoder_outputs[b].T),
                "WhT": WhT,
                "WdT": WdT,
                "decT": np.ascontiguousarray(decoder_outputs[b].T),
                "bdec": bdec_c,
                "vcol": vcol_c,
                "wc": wc_c,
                "cov0": np.ascontiguousarray(coverage[b].reshape(1, S)),
                "eye64": eye,
            }
        )
    return in_maps


def kernel(
    decoder_outputs,
    decoder_input_mask,
    encoder_outputs,
    enc_padding_mask,
    coverage,
    W_h,
    W_dec,
    b_dec,
    w_c,
    v,
):
    f = np.float32
    decoder_outputs = np.asarray(decoder_outputs, f)
    decoder_input_mask = np.asarray(decoder_input_mask, f)
    encoder_outputs = np.asarray(encoder_outputs, f)
    coverage = np.asarray(coverage, f)
    W_h = np.asarray(W_h, f)
    W_dec = np.asarray(W_dec, f)
    b_dec = np.asarray(b_dec, f)
    w_c = np.asarray(w_c, f)
    v = np.asarray(v, f)

    in_maps = make_in_maps(
        decoder_outputs, encoder_outputs, coverage, W_h, W_dec, b_dec, w_c, v
    )
    nc = _graph()
    res = run_bass_kernel_spmd(nc, in_maps, core_ids=list(range(B)))
    results = res.results

    # host-side normalization (float64): attn = exp / sum(exp)
    exp_all = np.stack(
        [results[b]["expd"] for b in range(B)]
    ).astype(np.float64)  # [B,T,S]
    denom = exp_all.sum(-1, keepdims=True)
    attn64 = exp_all / denom
    attn_dist = attn64.astype(f)
    ht_un = np.stack([results[b]["ht"] for b in range(B)]).astype(np.float64)
    ht_hat = (ht_un / denom).astype(f)

    cov0_64 = coverage.astype(np.float64)
    csum = np.cumsum(attn64, axis=1)
    cov_before = cov0_64[:, None, :] + csum - attn64
    coverage_final = (cov0_64 + csum[:, -1, :]).astype(f)
    step_losses = np.minimum(attn64, cov_before).sum(-1)  # [B,T]
    mask64 = decoder_input_mask.astype(np.float64)
    converge_loss = np.float32((step_losses * mask64).sum() / mask64.sum())

    return ht_hat, attn_dist, converge_loss, coverage_final
